# revision 8
# baseline (speedup 1.0000x reference)
"""Trainium2 Bass kernel for masked multi-head attention (B=4, S=1024, D=1024, H=16).

Sharding: 8 cores; core c handles batch b=c//2, query rows [r*512,(r+1)*512) with
r=c%2. No collectives: K/V projection work is duplicated within each core pair
(collectives on this stack cost ~175us each — far more than they would save).
All matmul operands are bf16 (converted on host): on TRN2 the PE streams bf16 at
2x the fp32r column rate, which halves every matmul stage; accumulation stays
fp32 in PSUM, so the end-to-end rel err is ~5e-3 against the fp32 reference
(harness gate 2e-2).

Layouts (per core), everything transposed on the host so contraction dims land on
SBUF partitions:
  xtq [D, 512]  = queries[b, rows].T          xtk/xtv [D, SK] = keys/values[b,:SK].T
  wq, wo [D, D] natural
  vmask [128, NK]: vmask[p,t] = 1.0 if t*128+p < valid_len[b] else 0.0

Pipeline: Q^T = Wq^T-tiles @ xtq; K^T likewise; V natural via lhsT=xtv tiles.
V is stored head-interleaved [sk, 16*(64+1)] with a vmask column per head: the
O^T = V_aug^T @ P^T matmul then yields both the attention output rows (0..63)
and the masked softmax denominator (row 64) in one accumulation. Scores are
computed transposed (S^T[sk, sq] = K_h^T-tile @ Q_h^T), exp(x/8) fused on
ScalarE while copying PSUM->SBUF (bf16 out), masking is purely multiplicative
via the zeroed V rows (exp(NEG)==0 in the reference, identical result).

Perf note: per-core PE cycle floor at the bf16 rate (~n/2 cycles per n-column
matmul) is ~131k cycles = ~46us for nk=6; measured 46.6us, i.e. the kernel is
at its structural roofline. Going lower needs fp8 (fails the 2e-2 error gate)
or cross-core dedup of the K/V projections (collectives cost ~175us on this
stack, so that path is a net loss).
"""

import os
import numpy as np

import concourse.bass as bass
import concourse.tile as tile
from concourse import bacc, mybir
from concourse.bass_utils import run_bass_kernel_spmd

B, S, D = 4, 1024, 1024
H, HD = 16, 64
N_CORES = 8
SQ = 512  # query rows per core
F32 = mybir.dt.float32
F32R = mybir.dt.float32r
BF16 = mybir.dt.bfloat16
NPBF16 = mybir.dt.np(mybir.dt.bfloat16)
VW = 65  # per-head v_store width (64 dims + 1 mask/ones column)

_module_cache: dict[int, object] = {}


def _build_module(nk: int, reps: int = 1, phases=None, variant: int = 0):
    """Build the SPMD Bass module; nk = number of 128-row key tiles.

    reps > 1 emits the whole pipeline multiple times (same pools) — used only
    for slope-based device-time measurement in the dev harness.
    """
    def on(name):
        return phases is None or name in phases

    chunk_pt = (nk >= 7) or (variant == 1)

    sk = nk * 128
    nkt = D // 128  # contraction k-tiles for the projections
    nm = D // 128   # output row-tiles (128 douts each)

    nc = bacc.Bacc("TRN2", target_bir_lowering=False, debug=False,
                   num_devices=N_CORES)

    xtq_d = nc.dram_tensor("xtq", [D, SQ], BF16, kind="ExternalInput")
    xtk_d = nc.dram_tensor("xtk", [D, sk], BF16, kind="ExternalInput")
    xtv_d = nc.dram_tensor("xtv", [D, sk], BF16, kind="ExternalInput")
    wq_d = nc.dram_tensor("wq", [D, D], BF16, kind="ExternalInput")
    wo_d = nc.dram_tensor("wo", [D, D], BF16, kind="ExternalInput")
    vm_d = nc.dram_tensor("vmask", [128, nk], F32, kind="ExternalInput")
    out_d = nc.dram_tensor("outT", [D, SQ], F32, kind="ExternalOutput")

    with tile.TileContext(nc) as tc:
        with (
            tc.tile_pool(name="w", bufs=1) as wpool,
            tc.tile_pool(name="xtq", bufs=1) as _xtqpool,
            tc.tile_pool(name="xtkv", bufs=(2 if variant == 1 else 1)) as xtkvpool,
            tc.tile_pool(name="qt", bufs=1) as qtpool,
            tc.tile_pool(name="kt", bufs=1) as ktpool,
            tc.tile_pool(name="vs", bufs=1) as vspool,
            tc.tile_pool(name="pt", bufs=(4 if ((nk >= 7) or (variant == 1)) else 2)) as ptpool,
            tc.tile_pool(name="ot", bufs=1) as otpool,
            tc.tile_pool(name="small", bufs=1) as smallpool,
            tc.tile_pool(name="inv", bufs=int(os.environ.get("K_INV", "2"))) as invpool,
            tc.tile_pool(name="dram", bufs=int(os.environ.get("K_DR", "2")), space="DRAM") as drampool,
            tc.tile_pool(name="psA", bufs=int(os.environ.get("K_PSA", "2")), space="PSUM") as psA,
            tc.tile_pool(name="psS", bufs=int(os.environ.get("K_PSS", "2")), space="PSUM") as psS,
            tc.tile_pool(name="psO", bufs=int(os.environ.get("K_PSO", "4")), space="PSUM") as psO,
        ):
          for _rep in range(reps):
              # ---- resident weights (wq slot later reused for wo via same tag)
              # wq/xtq k-slices interleaved so Q-proj's first accumulation
              # chain has its operands early.
              wq_sb = wpool.tile([128, nkt * D], BF16, tag="w")
              if variant == 1:
                  xtq_sb = xtkvpool.tile([128, nkt * SQ], BF16, tag="xtkv")
              else:
                  xtq_sb = _xtqpool.tile([128, nkt * SQ], BF16, tag="xtq")
              for k in range(nkt):
                  nc.sync.dma_start(out=wq_sb[:, k * D:(k + 1) * D],
                                    in_=wq_d.ap()[k * 128:(k + 1) * 128, :])
                  nc.sync.dma_start(out=xtq_sb[:, k * SQ:(k + 1) * SQ],
                                    in_=xtq_d.ap()[k * 128:(k + 1) * 128, :])

              vmask_sb = smallpool.tile([128, nk], F32, tag="vmask")
              nc.sync.dma_start(out=vmask_sb[:], in_=vm_d.ap())
              ones16 = smallpool.tile([128, 16], F32, tag="ones16")
              nc.vector.memset(ones16[:], 1.0)

              # ---- Q^T projection: qt[dout, sq], row-tile m on partitions
              qt_sb = qtpool.tile([128, nm * SQ], BF16, tag="qt")
              for m in range(nm if on("qt") else 0):
                  ps = psA.tile([128, SQ], F32, tag="proj")
                  for k in range(nkt):
                      nc.tensor.matmul(
                          ps[:],
                          wq_sb[:, k * D + m * 128: k * D + (m + 1) * 128],
                          xtq_sb[:, k * SQ:(k + 1) * SQ],
                          start=(k == 0), stop=(k == nkt - 1))
                  nc.vector.tensor_copy(qt_sb[:, m * SQ:(m + 1) * SQ], ps[:])

              # ---- K^T projection: kt[dout, sk]
              xtk_sb = xtkvpool.tile([128, nkt * sk], BF16, tag="xtkv")
              for k in range(nkt):
                  nc.sync.dma_start(out=xtk_sb[:, k * sk:(k + 1) * sk],
                                    in_=xtk_d.ap()[k * 128:(k + 1) * 128, :])
              kt_sb = ktpool.tile([128, nm * sk], BF16, tag="kt")
              nsplits = [(o, min(512, sk - o)) for o in range(0, sk, 512)]
              for m in range(nm if on("kt") else 0):
                  for (noff, nw) in nsplits:
                      ps = psA.tile([128, 512], F32, tag="proj")
                      for k in range(nkt):
                          nc.tensor.matmul(
                              ps[:, :nw],
                              wq_sb[:, k * D + m * 128: k * D + (m + 1) * 128],
                              xtk_sb[:, k * sk + noff: k * sk + noff + nw],
                              start=(k == 0), stop=(k == nkt - 1))
                      nc.vector.tensor_copy(
                          kt_sb[:, m * sk + noff: m * sk + noff + nw], ps[:, :nw])

              # ---- V projection into head-interleaved store with mask columns
              xtv_sb = xtkvpool.tile([128, nkt * sk], BF16, tag="xtkv")
              for k in range(nkt):
                  nc.sync.dma_start(out=xtv_sb[:, k * sk:(k + 1) * sk],
                                    in_=xtv_d.ap()[k * 128:(k + 1) * 128, :])
              vs_sb = vspool.tile([128, nk * H * VW], BF16, tag="vs")
              for t in range(nk if on("v") else 0):
                  for half in range(2):  # d columns [half*512, half*512+512)
                      ps = psA.tile([128, 512], F32, tag="proj")
                      for k in range(nkt):
                          nc.tensor.matmul(
                              ps[:],
                              xtv_sb[:, k * sk + t * 128: k * sk + (t + 1) * 128],
                              wq_sb[:, k * D + half * 512: k * D + half * 512 + 512],
                              start=(k == 0), stop=(k == nkt - 1))
                      dst = vs_sb[:, t * H * VW + half * 8 * VW:
                                  t * H * VW + (half + 1) * 8 * VW]
                      dst = dst.rearrange("p (h c) -> p h c", c=VW)[:, :, 0:HD]
                      src = ps[:].rearrange("p (h c) -> p h c", c=HD)
                      nc.vector.tensor_scalar_mul(dst, src, vmask_sb[:, t:t + 1])
                  mcols = vs_sb[:, t * H * VW: (t + 1) * H * VW]
                  mcols = mcols.rearrange("p (h c) -> p h c", c=VW)[:, :, HD:VW]
                  nc.vector.tensor_scalar_mul(
                      mcols, ones16[:].rearrange("p (h o) -> p h o", o=1),
                      vmask_sb[:, t:t + 1])

              # wo loads into the wq slot; Tile serializes on wq's last reader
              wo_sb = wpool.tile([128, nkt * D], BF16, tag="w")
              for k in range(nkt):
                  nc.sync.dma_start(out=wo_sb[:, k * D:(k + 1) * D],
                                    in_=wo_d.ap()[k * 128:(k + 1) * 128, :])

              # ---- attention per head
              ot_sb = otpool.tile([128, nm * SQ], BF16, tag="ot")
              for h in range(H if on("attn") else 0):
                  po = 64 * (h % 2)       # partition offset of this head's douts
                  mb = h // 2             # dout row-tile holding this head
                  if not chunk_pt:
                      # P^T per head resident; score/exp pass then O^T pass
                      pt = ptpool.tile([128, nk * SQ], BF16, tag="pt")
                      for t in range(nk):
                          ss = psS.tile([128, SQ], F32, tag="s")
                          nc.tensor.matmul(
                              ss[:],
                              kt_sb[po:po + 64, mb * sk + t * 128: mb * sk + (t + 1) * 128],
                              qt_sb[po:po + 64, mb * SQ:(mb + 1) * SQ],
                              start=True, stop=True)
                          nc.scalar.activation(pt[:, t * SQ:(t + 1) * SQ], ss[:],
                                               mybir.ActivationFunctionType.Exp,
                                               scale=0.125)
                      po_ps = psO.tile([VW, SQ], F32, tag="o")
                      for t in range(nk):
                          nc.tensor.matmul(
                              po_ps[:],
                              vs_sb[:, t * H * VW + h * VW: t * H * VW + (h + 1) * VW],
                              pt[:, t * SQ:(t + 1) * SQ],
                              start=(t == 0), stop=(t == nk - 1))
                  else:
                      # chunked P^T (smaller SBUF footprint for large nk)
                      po_ps = psO.tile([VW, SQ], F32, tag="o")
                      for t in range(nk):
                          ss = psS.tile([128, SQ], F32, tag="s")
                          nc.tensor.matmul(
                              ss[:],
                              kt_sb[po:po + 64, mb * sk + t * 128: mb * sk + (t + 1) * 128],
                              qt_sb[po:po + 64, mb * SQ:(mb + 1) * SQ],
                              start=True, stop=True)
                          ptc = ptpool.tile([128, SQ], BF16, tag="pt")
                          nc.scalar.activation(ptc[:], ss[:],
                                               mybir.ActivationFunctionType.Exp,
                                               scale=0.125)
                          nc.tensor.matmul(
                              po_ps[:],
                              vs_sb[:, t * H * VW + h * VW: t * H * VW + (h + 1) * VW],
                              ptc[:],
                              start=(t == 0), stop=(t == nk - 1),
                              skip_group_check=True)
                  inv = invpool.tile([1, SQ], F32, tag="inv")
                  nc.vector.reciprocal(inv[:], po_ps[64:65, :])
                  inv_dr = drampool.tile([1, SQ], F32, tag="invdr")
                  nc.sync.dma_start(out=inv_dr[:], in_=inv[:])
                  inv_rep = invpool.tile([64, SQ], F32, tag="invrep")
                  nc.sync.dma_start(out=inv_rep[:],
                                    in_=inv_dr[0:1, :].partition_broadcast(64))
                  nc.vector.tensor_mul(
                      ot_sb[po:po + 64, mb * SQ:(mb + 1) * SQ],
                      po_ps[0:64, :], inv_rep[:])

              # ---- output projection: outT[dout, sq] = Wo^T-tiles @ O^T
              for m in range(nm if on("out") else 0):
                  ps = psA.tile([128, SQ], F32, tag="proj")
                  for k in range(nkt):
                      nc.tensor.matmul(
                          ps[:],
                          wo_sb[:, k * D + m * 128: k * D + (m + 1) * 128],
                          ot_sb[:, k * SQ:(k + 1) * SQ],
                          start=(k == 0), stop=(k == nkt - 1))
                  osb = invpool.tile([128, SQ], F32, tag="outsb")
                  nc.vector.tensor_copy(osb[:], ps[:])
                  nc.sync.dma_start(out=out_d.ap()[m * 128:(m + 1) * 128, :],
                                    in_=osb[:])

    nc.compile()
    return nc


def make_in_maps(queries, keys, values, vls, W_q, W_o, nk):
    """Host staging: transpose + bf16-convert per-core inputs."""
    sk = nk * 128
    wq16 = np.ascontiguousarray(np.asarray(W_q, dtype=np.float32).astype(NPBF16))
    wo16 = np.ascontiguousarray(np.asarray(W_o, dtype=np.float32).astype(NPBF16))
    queries = np.asarray(queries, dtype=np.float32)
    keys = np.asarray(keys, dtype=np.float32)
    values = np.asarray(values, dtype=np.float32)
    in_maps = []
    for c in range(N_CORES):
        b, r = c // 2, c % 2
        vl = int(vls[b])
        vm = (np.arange(sk) < vl).astype(np.float32).reshape(nk, 128).T
        in_maps.append({
            "xtq": np.ascontiguousarray(
                queries[b, r * SQ:(r + 1) * SQ, :].T.astype(NPBF16)),
            "xtk": np.ascontiguousarray(keys[b, :sk, :].T.astype(NPBF16)),
            "xtv": np.ascontiguousarray(values[b, :sk, :].T.astype(NPBF16)),
            "wq": wq16,
            "wo": wo16,
            "vmask": np.ascontiguousarray(vm),
        })
    return in_maps


def kernel(queries, keys, values, valid_lengths, W_q, W_o):
    vls = np.asarray(valid_lengths).astype(np.int64)

    nk = max(1, int(-(-int(vls.max()) // 128)))  # ceil(max_vl/128)

    nc = _module_cache.get(nk)
    if nc is None:
        nc = _build_module(nk)
        _module_cache[nk] = nc

    in_maps = make_in_maps(queries, keys, values, vls, W_q, W_o, nk)

    res = run_bass_kernel_spmd(nc, in_maps, list(range(N_CORES)))

    out = np.empty((B, S, D), dtype=np.float32)
    for c in range(N_CORES):
        b, r = c // 2, c % 2
        out[b, r * SQ:(r + 1) * SQ, :] = res.results[c]["outT"].T
    return out

